# revision 25
# baseline (speedup 1.0000x reference)
"""Trainium2 Bass kernel for an 8-expert top-2 MoE layer (nn_MoE_21723944583386).

Strategy (expert-parallel, one expert per NeuronCore):
  * Host: noisy top-k gating in numpy (fp32, op-for-op identical to the
    jax reference — verified bit-identical top-2 routing), plus the tiny
    auxiliary-loss reductions.
  * Dispatch: for each expert, gather its routed tokens, pre-scale each
    token row by its gate value (relu(g*x@W1)@W2 == g*relu(x@W1)@W2 for
    g>0), transpose to feature-major and zero-pad to a common capacity.
  * Device (per core): yt = w2.T @ relu(w1.T @ xgt) — two chained
    matmul_tile_kernel calls inside one TileContext; weights stay in
    their natural [D,H]/[H,D] layouts, tokens stay transposed.
  * Combine: host scatter-adds each core's [D, C] output columns back
    into the [T, D] output.
"""

import os

import numpy as np

LAST_EXEC_NS = None
LAST_RESULTS = None

B, L, D, H, E, K = 2, 2048, 1024, 2048, 8, 2
NOISE_EPS = 0.01
CVLOSS, SWITCHLOSS, ZLOSS = 0.01, 0.1, 0.0001
N_CORES = 8
P = 128

# matmul precision mode:
#   "f32"  — full fp32 matmuls (4 cyc/row), rel err ~5e-7
#   "f32r" — fp32r / TF32-like (2 cyc/row measured), rel err ~2e-4
#   "bf16" — bfloat16 inputs (1 cyc/row + FWL), rel err ~2e-3
MODE = os.environ.get("MOE_MODE", "f32r")


def _blocks(C):
    """Split C token columns into near-equal moving blocks of <=512.

    Equal-ish sizes keep every block >=256 (full fp32r stream rate) and
    avoid tiny-matmul issue floors."""
    n = -(-C // 512)
    base, extra = divmod(C, n)
    out = []
    o = 0
    for i in range(n):
        b = base + (1 if i < extra else 0)
        out.append((o, b))
        o += b
    return out


def _build_expert_mlp(C: int):
    """Fused Bass program: yt[D,C] = w2[H,D].T @ relu(w1[D,H].T @ xgt[D,C]).

    h1 stays resident in SBUF; weights are streamed tile-by-tile (each
    weight byte is used exactly once); one weight load feeds all token
    blocks; relu eviction on the vector engine.
    """
    import concourse.mybir as mybir
    import concourse.tile as tile
    from concourse import bacc

    nc = bacc.Bacc(None, target_bir_lowering=False, debug=False)

    in_dt = {"f32": mybir.dt.float32,
             "f32r": mybir.dt.float32r,
             "bf16": mybir.dt.bfloat16}[MODE]
    h1_dt = in_dt
    blocks = _blocks(C)
    KD, KH = D // P, H // P      # 8 k-chunks (stage 1), 16 (stage 2)

    with tile.TileContext(nc) as tc:
        with tc.tile_pool(name="dram", bufs=1, space="DRAM") as dram, \
             tc.tile_pool(name="xres", bufs=1) as xres, \
             tc.tile_pool(name="h1res", bufs=1) as h1res, \
             tc.tile_pool(name="wpool", bufs=3) as wpool, \
             tc.tile_pool(name="psum", bufs=2, space="PSUM") as psum, \
             tc.tile_pool(name="ypool", bufs=4) as ypool:
            xgt = dram.tile([P, KD, C], in_dt, kind="ExternalInput",
                            name="xgt")
            w1 = dram.tile([P, KD, H], in_dt, kind="ExternalInput",
                           name="w1")
            w2 = dram.tile([P, KH, D], in_dt, kind="ExternalInput",
                           name="w2")
            yt = dram.tile([P, KD, C], mybir.dt.float32,
                           kind="ExternalOutput", name="yt")

            xsb = [xres.tile([P, C], in_dt, name=f"x{k}") for k in range(KD)]
            h1sb = [h1res.tile([P, C], h1_dt, name=f"h{m}")
                    for m in range(KH)]

            # stage 1: h1[m] = relu(sum_k w1[k,m].T @ x[k])
            for m in range(KH):
                ps = [psum.tile([P, b], mybir.dt.float32, tag=f"ps{i}",
                                name=f"ps{i}", bufs=(3 if b == 512 else 2))
                      for i, (_, b) in enumerate(blocks)]
                wt = wpool.tile([P, KD, P], in_dt, tag="w1", name="wt")
                nc.sync.dma_start(wt[:], w1[:, :, m * P:(m + 1) * P])
                for k in range(KD):
                    if m == 0:
                        nc.sync.dma_start(xsb[k][:], xgt[:, k, :])
                    for i, (o, b) in enumerate(blocks):
                        nc.tensor.matmul(ps[i][:], wt[:, k, :],
                                         xsb[k][:, o:o + b],
                                         start=(k == 0), stop=(k == KD - 1))
                for i, (o, b) in enumerate(blocks):
                    nc.vector.tensor_scalar_max(h1sb[m][:, o:o + b],
                                                ps[i][:], 0.0)

            # stage 2: yt[d] = sum_h w2[h,d].T @ h1[h]
            for d in range(KD):
                ps = [psum.tile([P, b], mybir.dt.float32, tag=f"ps{i}",
                                name=f"ps{i}", bufs=(3 if b == 512 else 2))
                      for i, (_, b) in enumerate(blocks)]
                wt = wpool.tile([P, KH, P], in_dt, tag="w2", name="wt")
                nc.sync.dma_start(wt[:], w2[:, :, d * P:(d + 1) * P])
                for h in range(KH):
                    for i, (o, b) in enumerate(blocks):
                        nc.tensor.matmul(ps[i][:], wt[:, h, :],
                                         h1sb[h][:, o:o + b],
                                         start=(h == 0), stop=(h == KH - 1))
                yo = ypool.tile([P, C], mybir.dt.float32, tag="y",
                                name="yo")
                for i, (o, b) in enumerate(blocks):
                    nc.vector.tensor_copy(yo[:, o:o + b], ps[i][:])
                nc.sync.dma_start(yt[:, d, :], yo[:])
    nc.compile()
    names = {"xgt": xgt.name, "w1": w1.name, "w2": w2.name, "yt": yt.name}
    return nc, names


def _round_f32r(a: np.ndarray) -> np.ndarray:
    """Round fp32 to fp32r (11-bit mantissa, low 12 bits zero), RNE."""
    u = np.ascontiguousarray(a, dtype=np.float32).view(np.uint32)
    u64 = u.astype(np.uint64)
    r = (u64 + 0x7FF + ((u64 >> 12) & 1)) & 0xFFFFF000
    return r.astype(np.uint32).view(np.float32)


def _part3d(a: np.ndarray) -> np.ndarray:
    """[(m p), n] -> [p, m, n] with p=128 (partition-major DRAM layout)."""
    m, n = a.shape
    return np.ascontiguousarray(
        a.reshape(m // P, P, n).transpose(1, 0, 2))


def _unpart3d(a: np.ndarray) -> np.ndarray:
    """[p, m, n] -> [(m p), n]."""
    p, m, n = a.shape
    return a.transpose(1, 0, 2).reshape(m * p, n)


def kernel(x, w_gate, w1, w2, eps):
    x = np.asarray(x, dtype=np.float32)
    w_gate = np.asarray(w_gate, dtype=np.float32)
    w1 = np.asarray(w1, dtype=np.float32)
    w2 = np.asarray(w2, dtype=np.float32)
    eps = np.asarray(eps, dtype=np.float32)

    T = B * L
    xf = x.reshape(T, D)

    # ---- gating (host, fp32 to match the jax reference) ----
    lin = xf @ w_gate                       # [T, 2E]
    clean, raw = lin[:, :E], lin[:, E:]
    # jax.nn.softplus: stable log1p(exp(x))
    softplus = np.logaddexp(0.0, raw).astype(np.float32)
    noise_std = softplus + np.float32(NOISE_EPS)
    logits = clean + eps * noise_std
    lmax = logits.max(axis=1, keepdims=True)
    ex = np.exp(logits - lmax)
    exsum = ex.sum(axis=1, keepdims=True)
    probs = ex / exsum                      # [T, E]

    order = np.argsort(-probs, axis=1, kind="stable")
    top_i = order[:, :K]                    # [T, K]
    rows = np.arange(T)[:, None]
    top_g = probs[rows, top_i]              # [T, K]
    gates = np.zeros_like(probs)
    gates[rows, top_i] = top_g

    # ---- aux loss (host, float64 reductions of [T,E] stats) ----
    gates64 = gates.astype(np.float64)
    probs64 = probs.astype(np.float64)
    expert_size = (gates64 > 0).sum(0)
    g_sum = gates64.sum(0)
    g_norm = g_sum / g_sum.sum()
    cv = g_norm.var(ddof=1) / (g_norm.mean() ** 2 + 1e-10)
    p_norm = probs64.sum(0) / probs64.sum()
    f_norm = expert_size / expert_size.sum()
    switch = (1.0 - (p_norm * f_norm).sum()) * E
    lse = (lmax[:, 0] + np.log(exsum[:, 0])).astype(np.float64)
    zl = np.mean(lse ** 2)
    loss = np.float32(CVLOSS * cv + SWITCHLOSS * switch + ZLOSS * zl)

    # ---- dispatch: gather per-expert token batches, gate-scaled ----
    tok_idx = []        # token index per expert slot
    counts = np.zeros(E, dtype=np.int64)
    for e in range(E):
        te, ke = np.nonzero(top_i == e)
        tok_idx.append((te, top_g[te, ke]))
        counts[e] = te.size
    C = int(np.ceil(max(8, counts.max()) / 8) * 8)

    nc, names = _build_expert_mlp(C)

    if MODE == "f32r":
        conv = _round_f32r
    elif MODE == "bf16":
        import ml_dtypes
        conv = lambda a: np.asarray(a, dtype=ml_dtypes.bfloat16)
    else:
        conv = lambda a: a

    in_maps = []
    for e in range(E):
        te, g = tok_idx[e]
        xg = np.zeros((C, D), dtype=np.float32)
        xg[: te.size] = xf[te] * g[:, None]
        in_maps.append({
            names["xgt"]: _part3d(conv(np.ascontiguousarray(xg.T))),
            names["w1"]: _part3d(conv(w1[e])),
            names["w2"]: _part3d(conv(w2[e])),
        })

    # ---- device: one expert MLP per core ----
    from concourse.bass_utils import run_bass_kernel_spmd

    trace = os.environ.get("MOE_TRACE") == "1"
    kwargs = {}
    if trace:
        kwargs = {"trace": True, "trace_cores": list(range(N_CORES))}
    res = run_bass_kernel_spmd(nc, in_maps, core_ids=list(range(N_CORES)),
                               **kwargs)
    global LAST_EXEC_NS, LAST_RESULTS
    LAST_RESULTS = res
    if res.exec_time_ns is not None:
        LAST_EXEC_NS = res.exec_time_ns
    elif res.per_core_scope_times:
        LAST_EXEC_NS = max(
            max(d.values()) for d in res.per_core_scope_times.values())

    # ---- combine: scatter-add per-expert outputs ----
    y = np.zeros((T, D), dtype=np.float32)
    for e in range(E):
        te, _ = tok_idx[e]
        yt = _unpart3d(res.results[e][names["yt"]])     # [D, C]
        y[te] += yt[:, : te.size].T
    return y.reshape(B, L, D), loss


# revision 26
# speedup vs baseline: 1.0191x; 1.0191x over previous
"""Trainium2 Bass kernel for an 8-expert top-2 MoE layer (nn_MoE_21723944583386).

Strategy (expert-parallel, one expert per NeuronCore):
  * Host: noisy top-k gating in numpy (fp32, op-for-op identical to the
    jax reference — verified bit-identical top-2 routing), plus the tiny
    auxiliary-loss reductions.
  * Dispatch: for each expert, gather its routed tokens, pre-scale each
    token row by its gate value (relu(g*x@W1)@W2 == g*relu(x@W1)@W2 for
    g>0), transpose to feature-major and zero-pad to a common capacity.
  * Device (per core): yt = w2.T @ relu(w1.T @ xgt) — two chained
    matmul_tile_kernel calls inside one TileContext; weights stay in
    their natural [D,H]/[H,D] layouts, tokens stay transposed.
  * Combine: host scatter-adds each core's [D, C] output columns back
    into the [T, D] output.
"""

import os

import numpy as np

LAST_EXEC_NS = None
LAST_RESULTS = None

B, L, D, H, E, K = 2, 2048, 1024, 2048, 8, 2
NOISE_EPS = 0.01
CVLOSS, SWITCHLOSS, ZLOSS = 0.01, 0.1, 0.0001
N_CORES = 8
P = 128

# matmul precision mode:
#   "f32"  — full fp32 matmuls (4 cyc/row), rel err ~5e-7
#   "f32r" — fp32r / TF32-like (2 cyc/row measured), rel err ~2e-4
#   "bf16" — bfloat16 inputs (1 cyc/row + FWL), rel err ~2e-3
MODE = os.environ.get("MOE_MODE", "f32r")


def _blocks(C):
    """Split C token columns into near-equal moving blocks of <=512.

    Equal-ish sizes keep every block >=256 (full fp32r stream rate) and
    avoid tiny-matmul issue floors."""
    n = -(-C // 512)
    base = (C // n) // 8 * 8
    out = []
    o = 0
    for i in range(n):
        b = base if i < n - 1 else C - o
        out.append((o, b))
        o += b
    return out


def _build_expert_mlp(C: int):
    """Fused Bass program: yt[D,C] = w2[H,D].T @ relu(w1[D,H].T @ xgt[D,C]).

    h1 stays resident in SBUF; weights are streamed tile-by-tile (each
    weight byte is used exactly once); one weight load feeds all token
    blocks; relu eviction on the vector engine.
    """
    import concourse.mybir as mybir
    import concourse.tile as tile
    from concourse import bacc

    nc = bacc.Bacc(None, target_bir_lowering=False, debug=False)

    in_dt = {"f32": mybir.dt.float32,
             "f32r": mybir.dt.float32r,
             "bf16": mybir.dt.bfloat16}[MODE]
    h1_dt = in_dt
    blocks = _blocks(C)
    KD, KH = D // P, H // P      # 8 k-chunks (stage 1), 16 (stage 2)

    with tile.TileContext(nc) as tc:
        with tc.tile_pool(name="dram", bufs=1, space="DRAM") as dram, \
             tc.tile_pool(name="xres", bufs=1) as xres, \
             tc.tile_pool(name="h1res", bufs=1) as h1res, \
             tc.tile_pool(name="wpool", bufs=3) as wpool, \
             tc.tile_pool(name="psum", bufs=2, space="PSUM") as psum, \
             tc.tile_pool(name="ypool", bufs=4) as ypool:
            xgt = dram.tile([P, KD, C], in_dt, kind="ExternalInput",
                            name="xgt")
            w1 = dram.tile([P, KD, H], in_dt, kind="ExternalInput",
                           name="w1")
            w2 = dram.tile([P, KH, D], in_dt, kind="ExternalInput",
                           name="w2")
            yt = dram.tile([P, KD, C], mybir.dt.float32,
                           kind="ExternalOutput", name="yt")

            xsb = [xres.tile([P, C], in_dt, name=f"x{k}") for k in range(KD)]
            h1sb = [h1res.tile([P, C], h1_dt, name=f"h{m}")
                    for m in range(KH)]

            # stage 1: h1[m] = relu(sum_k w1[k,m].T @ x[k])
            for m in range(KH):
                ps = [psum.tile([P, b], mybir.dt.float32, tag=f"ps{i}",
                                name=f"ps{i}", bufs=(3 if b == 512 else 2))
                      for i, (_, b) in enumerate(blocks)]
                wt = wpool.tile([P, KD, P], in_dt, tag="w1", name="wt")
                nc.sync.dma_start(wt[:], w1[:, :, m * P:(m + 1) * P])
                for k in range(KD):
                    if m == 0:
                        nc.sync.dma_start(xsb[k][:], xgt[:, k, :])
                    for i, (o, b) in enumerate(blocks):
                        nc.tensor.matmul(ps[i][:], wt[:, k, :],
                                         xsb[k][:, o:o + b],
                                         start=(k == 0), stop=(k == KD - 1))
                for i, (o, b) in enumerate(blocks):
                    nc.vector.tensor_scalar_max(h1sb[m][:, o:o + b],
                                                ps[i][:], 0.0)

            # stage 2: yt[d] = sum_h w2[h,d].T @ h1[h]
            for d in range(KD):
                ps = [psum.tile([P, b], mybir.dt.float32, tag=f"ps{i}",
                                name=f"ps{i}", bufs=(3 if b == 512 else 2))
                      for i, (_, b) in enumerate(blocks)]
                wt = wpool.tile([P, KH, P], in_dt, tag="w2", name="wt")
                nc.sync.dma_start(wt[:], w2[:, :, d * P:(d + 1) * P])
                for h in range(KH):
                    for i, (o, b) in enumerate(blocks):
                        nc.tensor.matmul(ps[i][:], wt[:, h, :],
                                         h1sb[h][:, o:o + b],
                                         start=(h == 0), stop=(h == KH - 1))
                yo = ypool.tile([P, C], mybir.dt.float32, tag="y",
                                name="yo")
                for i, (o, b) in enumerate(blocks):
                    nc.vector.tensor_copy(yo[:, o:o + b], ps[i][:])
                nc.sync.dma_start(yt[:, d, :], yo[:])
    nc.compile()
    names = {"xgt": xgt.name, "w1": w1.name, "w2": w2.name, "yt": yt.name}
    return nc, names


def _round_f32r(a: np.ndarray) -> np.ndarray:
    """Round fp32 to fp32r (11-bit mantissa, low 12 bits zero), RNE."""
    u = np.ascontiguousarray(a, dtype=np.float32).view(np.uint32)
    u64 = u.astype(np.uint64)
    r = (u64 + 0x7FF + ((u64 >> 12) & 1)) & 0xFFFFF000
    return r.astype(np.uint32).view(np.float32)


def _part3d(a: np.ndarray) -> np.ndarray:
    """[(m p), n] -> [p, m, n] with p=128 (partition-major DRAM layout)."""
    m, n = a.shape
    return np.ascontiguousarray(
        a.reshape(m // P, P, n).transpose(1, 0, 2))


def _unpart3d(a: np.ndarray) -> np.ndarray:
    """[p, m, n] -> [(m p), n]."""
    p, m, n = a.shape
    return a.transpose(1, 0, 2).reshape(m * p, n)


def kernel(x, w_gate, w1, w2, eps):
    x = np.asarray(x, dtype=np.float32)
    w_gate = np.asarray(w_gate, dtype=np.float32)
    w1 = np.asarray(w1, dtype=np.float32)
    w2 = np.asarray(w2, dtype=np.float32)
    eps = np.asarray(eps, dtype=np.float32)

    T = B * L
    xf = x.reshape(T, D)

    # ---- gating (host, fp32 to match the jax reference) ----
    lin = xf @ w_gate                       # [T, 2E]
    clean, raw = lin[:, :E], lin[:, E:]
    # jax.nn.softplus: stable log1p(exp(x))
    softplus = np.logaddexp(0.0, raw).astype(np.float32)
    noise_std = softplus + np.float32(NOISE_EPS)
    logits = clean + eps * noise_std
    lmax = logits.max(axis=1, keepdims=True)
    ex = np.exp(logits - lmax)
    exsum = ex.sum(axis=1, keepdims=True)
    probs = ex / exsum                      # [T, E]

    order = np.argsort(-probs, axis=1, kind="stable")
    top_i = order[:, :K]                    # [T, K]
    rows = np.arange(T)[:, None]
    top_g = probs[rows, top_i]              # [T, K]
    gates = np.zeros_like(probs)
    gates[rows, top_i] = top_g

    # ---- aux loss (host, float64 reductions of [T,E] stats) ----
    gates64 = gates.astype(np.float64)
    probs64 = probs.astype(np.float64)
    expert_size = (gates64 > 0).sum(0)
    g_sum = gates64.sum(0)
    g_norm = g_sum / g_sum.sum()
    cv = g_norm.var(ddof=1) / (g_norm.mean() ** 2 + 1e-10)
    p_norm = probs64.sum(0) / probs64.sum()
    f_norm = expert_size / expert_size.sum()
    switch = (1.0 - (p_norm * f_norm).sum()) * E
    lse = (lmax[:, 0] + np.log(exsum[:, 0])).astype(np.float64)
    zl = np.mean(lse ** 2)
    loss = np.float32(CVLOSS * cv + SWITCHLOSS * switch + ZLOSS * zl)

    # ---- dispatch: gather per-expert token batches, gate-scaled ----
    tok_idx = []        # token index per expert slot
    counts = np.zeros(E, dtype=np.int64)
    for e in range(E):
        te, ke = np.nonzero(top_i == e)
        tok_idx.append((te, top_g[te, ke]))
        counts[e] = te.size
    C = int(np.ceil(max(8, counts.max()) / 8) * 8)

    nc, names = _build_expert_mlp(C)

    if MODE == "f32r":
        conv = _round_f32r
    elif MODE == "bf16":
        import ml_dtypes
        conv = lambda a: np.asarray(a, dtype=ml_dtypes.bfloat16)
    else:
        conv = lambda a: a

    in_maps = []
    for e in range(E):
        te, g = tok_idx[e]
        xg = np.zeros((C, D), dtype=np.float32)
        xg[: te.size] = xf[te] * g[:, None]
        in_maps.append({
            names["xgt"]: _part3d(conv(np.ascontiguousarray(xg.T))),
            names["w1"]: _part3d(conv(w1[e])),
            names["w2"]: _part3d(conv(w2[e])),
        })

    # ---- device: one expert MLP per core ----
    from concourse.bass_utils import run_bass_kernel_spmd

    trace = os.environ.get("MOE_TRACE") == "1"
    kwargs = {}
    if trace:
        kwargs = {"trace": True, "trace_cores": list(range(N_CORES))}
    res = run_bass_kernel_spmd(nc, in_maps, core_ids=list(range(N_CORES)),
                               **kwargs)
    global LAST_EXEC_NS, LAST_RESULTS
    LAST_RESULTS = res
    if res.exec_time_ns is not None:
        LAST_EXEC_NS = res.exec_time_ns
    elif res.per_core_scope_times:
        LAST_EXEC_NS = max(
            max(d.values()) for d in res.per_core_scope_times.values())

    # ---- combine: scatter-add per-expert outputs ----
    y = np.zeros((T, D), dtype=np.float32)
    for e in range(E):
        te, _ = tok_idx[e]
        yt = _unpart3d(res.results[e][names["yt"]])     # [D, C]
        y[te] += yt[:, : te.size].T
    return y.reshape(B, L, D), loss
